# revision 21
# baseline (speedup 1.0000x reference)
"""Trainium2 Bass kernel for nn_BaseAggregator_31439160607279.

Math (reference):
  af (a,c,f,t), imf (v,c,h,w), split c into k=2 heads of 256 ch.
  sims[a,v,k,hw,t] = sum_c af*imf ; + cls[a,v,k] ; relu ; max over hw ;
  masked mean over t (mask m[a,t] in {0,1}, den = f*sum_t m) ; sum over k.

Strategy:
  - Shard the image dim v=32 across 8 cores (4 images/core); audio replicated.
  - Pack ALL mask-active (a, t) pairs into the matmul M dim -> ~3219 rows ->
    26 M-tiles of 128.
  - Big matmuls in fp16: lhsT = packed audio rows (K=128 chunk, M=128),
    rhs = [imf pair | imf pair] (K=128, N=392), 2-chunk accumulate; per
    (mt, head) one 2-bank PSUM tile.
  - hw-max split across engines (the old all-DVE version was the pacer):
      k0 group: DVE reduce_max direct on PSUM fp32 (1x rate).
      k1 group: Scalar engine copies PSUM -> SBUF fp16 (1.2 GHz), then DVE
      reduce_max on fp16 SBUF runs in 2x_1P mode (2 elem/cycle).
  - Host precomputes cls_sims (tiny einsum) -> DMA'd as packed per-row bias;
    host also folds 1/(f*sum(m)) into the masked-sum one-hot weights, so the
    device does: add cls (gpsimd), relu (scalar), masked-sum matmul (PE,
    accumulated in one PSUM bank across all M-tiles), head-sum, out.
"""

import math
from contextlib import ExitStack

import numpy as np

import concourse.bacc as bacc
import concourse.mybir as mybir
import concourse.tile as tile
from concourse.bass_utils import run_bass_kernel_spmd

# Problem dims (hardcoded per spec)
A, V, C, F, T, H, W = 32, 32, 512, 1, 200, 14, 14
K = 2                    # heads
NCH = C // K             # 256 channels per head
KC = 2                   # channel chunks per head
KP = NCH // KC           # 128 = contraction per matmul
HW = H * W               # 196
HW2 = HW // 2            # 98
NCORES = 8
VL = V // NCORES         # 4 local images per core
NVP = VL // 2            # 2 local image pairs
NPAIR = 2 * HW           # 392 = matmul free dim per image pair

AFP_CHUNK = 7            # M-tiles per audio DMA chunk

TRACE = False
LAST_RESULTS = None

_kernel_cache = {}

f32 = mybir.dt.float32
f16 = mybir.dt.float16
X = mybir.AxisListType.X


def _build(MT: int):
    """Build + compile the per-core Bass program for MT packed-row tiles."""
    nc = bacc.Bacc("TRN2", target_bir_lowering=False, debug=False)

    # afp laid out mt-major: (KP, MT*K*KC*128) so each DMA chunk (a span of
    # M-tiles) is ONE transfer = 128 fat descriptors
    afp_d = nc.dram_tensor("afp", (KP, MT * K * KC * 128), f16, kind="ExternalInput")
    imf_d = nc.dram_tensor("imf", (KP, K * KC * VL * HW), f16, kind="ExternalInput")
    # maskcs = per-row one-hot audio columns pre-scaled by 1/(F*sum_t m[a])
    maskcs_d = nc.dram_tensor("maskcs", (KP, MT * A), f16, kind="ExternalInput")
    # clsb = per-row cls bias, packed (128, MT*K*VL)
    clsb_d = nc.dram_tensor("clsb", (KP, MT * K * VL), f16, kind="ExternalInput")
    outk_d = nc.dram_tensor("outk", (A, K * VL), f32, kind="ExternalOutput")
    outsum_d = nc.dram_tensor("outsum", (A, VL), f32, kind="ExternalOutput")

    with tile.TileContext(nc) as tc, ExitStack() as ctx:
        cst = ctx.enter_context(tc.tile_pool(name="cst", bufs=1))
        ps_big = ctx.enter_context(tc.tile_pool(name="ps_big", bufs=3, space="PSUM"))
        ps_num = ctx.enter_context(tc.tile_pool(name="ps_num", bufs=1, space="PSUM"))
        sm_pool = ctx.enter_context(tc.tile_pool(name="sm", bufs=3))

        # --- persistent SBUF tiles ---
        afp_sb = cst.tile([KP, MT * K * KC * 128], f16, tag="afp", name="afp_sb")

        def afp_lhs(mt, k, kc):
            off = ((mt * K + k) * KC + kc) * 128
            return afp_sb[:, off:off + 128]

        imf_sb = cst.tile([KP, K * KC * VL * HW], f16, tag="imf", name="imf_sb")
        maskcs_sb = cst.tile([KP, MT * A], f16, tag="maskcs", name="maskcs_sb")
        clsb_sb = cst.tile([KP, MT * K * VL], f16, tag="clsb", name="clsb_sb")

        def imf_rhs(k, kc, sub):
            off = (k * KC + kc) * (VL * HW) + sub * NPAIR
            return imf_sb[:, off:off + NPAIR]

        def maskc_lhs(mt):
            off = mt * A
            return maskcs_sb[:, off:off + A]

        # DMA order: compute-gating transfers first.
        half = K * KC * VL * HW // 2
        nc.sync.dma_start(out=imf_sb[:, 0:half], in_=imf_d.ap()[:, 0:half])
        # few fat chunks: DMA cost is ~67ns per (transfer x partition)
        # descriptor; mt-major afp layout makes each chunk ONE transfer
        afp_cuts = sorted(set([0, min(2, MT), min(8, MT), MT]))
        MTW = K * KC * 128                 # afp cols per M-tile

        def afp_chunk_dma(lo, hi):
            sl = slice(lo * MTW, hi * MTW)
            nc.sync.dma_start(out=afp_sb[:, sl], in_=afp_d.ap()[:, sl])

        afp_chunk_dma(afp_cuts[0], afp_cuts[1])
        nc.sync.dma_start(out=imf_sb[:, half:2 * half], in_=imf_d.ap()[:, half:2 * half])
        chunks = list(zip(afp_cuts[1:-1], afp_cuts[2:]))
        if chunks:
            afp_chunk_dma(*chunks[0])
        nc.sync.dma_start(out=clsb_sb[:], in_=clsb_d.ap())
        nc.sync.dma_start(out=maskcs_sb[:], in_=maskcs_d.ap())
        for lo, hi in chunks[1:]:
            afp_chunk_dma(lo, hi)

        # --- PE warm-up on a dedicated PSUM bank (doesn't cycle ps_big):
        # nudges the HAM clock-gate while the first DMA chunk lands ---
        ps_warm = ctx.enter_context(tc.tile_pool(name="ps_warm", bufs=1, space="PSUM"))
        warm = cst.tile([KP, 512], f16, tag="warm", name="warm_sb")
        nc.gpsimd.memset(warm[:], 0.0)
        pw = ps_warm.tile([128, 512], f32, tag="ps_warm", name="ps_warm")
        for w in range(3):
            nc.tensor.matmul(pw[:], lhsT=warm[:, 0:128], rhs=warm[:],
                             start=True, stop=True)

        # --- main loop over M-tiles ---
        num_ps = ps_num.tile([A, K * VL], f32, tag="ps_num", name="ps_numacc")
        smraw_tiles = []   # [128, K*VL] f16: per-row, per-(k,img) hw-max
        smcp_tiles = []    # [128, 1176] f16: ACT-copied images (k0 i2,i3 + k1)
        tt2_tiles = []     # [128, 294] f16: after two TT-max levels
        sm3_tiles = []

        def emit_group(mt, k):
            """4 matmuls for one (mt, head) -> one 2-bank PSUM tile.
            Returns the psum tile (consumed by DVE reduce or ACT copy)."""
            ps = ps_big.tile([128, 1024], f32, tag="ps_big", name="ps_sims")
            for sub in range(NVP):
                for kc in range(KC):
                    nc.tensor.matmul(
                        ps[:, sub * 512:sub * 512 + NPAIR],
                        lhsT=afp_lhs(mt, k, kc),
                        rhs=imf_rhs(k, kc, sub),
                        start=(kc == 0), stop=(kc == 1),
                    )
            return ps

        def psum_4d(ps):
            rv = ps[:].rearrange("p (b q) -> p b q", b=2)[:, :, 0:NPAIR]
            return rv.rearrange("p b (i x) -> p b i x", i=2)

        NCP = 6                  # images routed through the ACT-copy/TT path
        CPW = NCP * HW           # 1176 smcp cols

        def emit_reduce_k0b0(mt, ps):
            # DVE direct reduce of k0 bank0 (imgs 0,1): [p,2,196] -> [p,2]
            rv = ps[:, 0:NPAIR].rearrange("p (i x) -> p i x", i=2)
            nc.vector.reduce_max(smraw_tiles[mt][:, 0:2], rv, axis=X)

        def get_smcp(mt):
            while len(smcp_tiles) <= mt:
                smcp_tiles.append(
                    sm_pool.tile([128, CPW], f16, tag="smcp", name="smcp", bufs=4))
            return smcp_tiles[mt]

        def emit_copy_k0b1(mt, ps):
            # ACT copies k0 bank1 (imgs 2,3) -> smcp cols 0:392 (fp16)
            dst = get_smcp(mt)[:, 0:NPAIR].rearrange("p (i x) -> p i x", i=2)
            src = ps[:, 512:512 + NPAIR].rearrange("p (i x) -> p i x", i=2)
            nc.scalar.copy(dst, src)

        def emit_copy_k1(mt, ps):
            # ACT copies the whole k1 group (4 imgs) -> smcp cols 392:1176
            dst = get_smcp(mt)[:, NPAIR:CPW].rearrange(
                "p (b i x) -> p b i x", b=2, i=2)
            nc.scalar.copy(dst, psum_4d(ps))

        def emit_tt_tree(mt):
            # DVE fp16 TT-max level at 2x_1P: [p,6,196] -> [p,6,98]
            smcp = smcp_tiles[mt][:].rearrange("p (i x) -> p i x", i=NCP)
            t1 = sm_pool.tile([128, NCP * HW2], f16, tag="tt1", name="tt1", bufs=3)
            nc.vector.tensor_max(
                t1[:].rearrange("p (i x) -> p i x", i=NCP),
                smcp[:, :, 0:HW2], smcp[:, :, HW2:HW])
            tt2_tiles.append(t1)

        def emit_sbuf_reduce(mt):
            # DVE fp16 reduce: [p,6,98] -> [p,6] = (k0 i2,i3, k1 i0..3)
            nc.vector.reduce_max(
                smraw_tiles[mt][:, 2:K * VL],
                tt2_tiles[mt][:].rearrange("p (i x) -> p i x", i=NCP),
                axis=X,
            )

        def emit_addrelu(j):
            sm2 = sm_pool.tile([128, K * VL], f16, tag="sm2", name="sm2", bufs=3)
            nc.gpsimd.tensor_add(sm2[:], smraw_tiles[j][:],
                                 clsb_sb[:, j * K * VL:(j + 1) * K * VL])
            sm3 = sm_pool.tile([128, K * VL], f16, tag="sm3", name="sm3", bufs=8)
            nc.gpsimd.tensor_scalar_max(sm3[:], sm2[:], 0.0)
            sm3_tiles.append(sm3)

        def emit_numdot(j):
            nc.tensor.matmul(num_ps[:], lhsT=maskc_lhs(j), rhs=sm3_tiles[j][:],
                             start=(j == 0), stop=(j == MT - 1))

        for mt in range(MT):
            smraw = sm_pool.tile([128, K * VL], f16, tag="smraw", name="smraw", bufs=6)
            smraw_tiles.append(smraw)
            # k1 groups run one M-tile behind k0 so mt0-k1 never waits on the
            # second half of the image DMA
            last = mt == MT - 1
            ps0 = emit_group(mt, 0)
            if last and MT > 1:
                # shorten the drain: last mt reduces PSUM directly on DVE
                nc.vector.reduce_max(smraw_tiles[mt][:, 0:VL], psum_4d(ps0), axis=X)
            else:
                emit_reduce_k0b0(mt, ps0)
                emit_copy_k0b1(mt, ps0)
            if mt >= 1:
                ps1 = emit_group(mt - 1, 1)
                emit_copy_k1(mt - 1, ps1)
            if last:
                ps1 = emit_group(mt, 1)
                if MT > 1:
                    nc.vector.reduce_max(smraw_tiles[mt][:, VL:2 * VL],
                                         psum_4d(ps1), axis=X)
                else:
                    emit_copy_k1(mt, ps1)
            # pipeline: tt + sbuf-reduce 2 behind, add/relu 3, numdot 5
            if mt >= 2:
                emit_tt_tree(mt - 2)
                emit_sbuf_reduce(mt - 2)
            if mt >= 3:
                emit_addrelu(mt - 3)
            if mt >= 5:
                emit_numdot(mt - 5)

        NDIR = 1 if MT > 1 else 0       # last mt bypasses the copy path
        for j in range(max(MT - 2, 0), MT - NDIR):
            emit_tt_tree(j)
            emit_sbuf_reduce(j)
        for j in range(max(MT - 3, 0), MT):
            emit_addrelu(j)
        for j in range(max(MT - 5, 0), MT):
            emit_numdot(j)

        # --- outputs: num is already scaled by 1/(F*sum m) on the host side ---
        outk_sb = cst.tile([A, K * VL], f32, tag="outk", name="outk_sb")
        nc.vector.tensor_copy(outk_sb[:], num_ps[:])
        outsum_sb = cst.tile([A, VL], f32, tag="outsum", name="outsum_sb")
        nc.vector.tensor_add(outsum_sb[:], outk_sb[:, 0:VL], outk_sb[:, VL:2 * VL])
        nc.sync.dma_start(out=outk_d.ap(), in_=outk_sb[:])
        nc.sync.dma_start(out=outsum_d.ap(), in_=outsum_sb[:])

    nc.compile()
    return nc


def prepare_inputs(audio_feats, image_feats, audio_cls, image_cls, audio_mask):
    """Host-side shard + layout prep. Returns (MT, in_maps)."""
    af = np.ascontiguousarray(audio_feats, dtype=np.float32).reshape(A, K, KC, KP, T)
    imf = np.ascontiguousarray(image_feats, dtype=np.float32).reshape(V, K, KC, KP, HW)
    acls = np.ascontiguousarray(audio_cls, dtype=np.float32).reshape(A, K, NCH)
    icls = np.ascontiguousarray(image_cls, dtype=np.float32).reshape(V, K, NCH)
    mask = np.asarray(audio_mask)

    rows_a, rows_t = np.nonzero(mask != 0)
    L = len(rows_a)
    MT = max(1, math.ceil(L / 128))
    LP = MT * 128

    # audio rows, shared by all cores: (K, KC, KP, MT*128) fp16
    af_rows = np.zeros((LP, K, KC, KP), np.float32)
    af_rows[:L] = af[rows_a, :, :, :, rows_t]
    # mt-major layout: (KP, MT*K*KC*128)
    afp = np.ascontiguousarray(
        af_rows.reshape(MT, 128, K, KC, KP).transpose(4, 0, 2, 3, 1)
        .reshape(KP, MT * K * KC * 128)
    ).astype(np.float16)

    # one-hot audio columns pre-scaled by 1/(F * sum_t m[a])
    msum = mask.astype(np.float64).sum(1)
    inv_den = np.where(msum > 0, 1.0 / (F * np.maximum(msum, 1e-30)), 0.0)
    mc = np.zeros((LP, A), np.float16)
    mc[np.arange(L), rows_a] = inv_den[rows_a].astype(np.float16)
    maskcs = mc.reshape(MT, 128, A).transpose(1, 0, 2).reshape(128, MT * A)
    maskcs = np.ascontiguousarray(maskcs)

    # host cls similarity: (A, V, K)
    cls = np.einsum("akc,vkc->avk", acls, icls).astype(np.float32)

    in_maps = []
    for ci in range(NCORES):
        vsl = slice(ci * VL, (ci + 1) * VL)
        imf_h = np.ascontiguousarray(
            imf[vsl].transpose(3, 1, 2, 0, 4).reshape(KP, K * KC * VL * HW)
        ).astype(np.float16)
        # per-row cls bias, packed like smraw: [row, k, v_local]
        clsb_rows = np.zeros((LP, K, VL), np.float32)
        clsb_rows[:L] = cls[rows_a][:, vsl, :].transpose(0, 2, 1)
        clsb = np.ascontiguousarray(
            clsb_rows.reshape(MT, 128, K * VL).transpose(1, 0, 2)
            .reshape(128, MT * K * VL)
        ).astype(np.float16)
        in_maps.append({
            "afp": afp,
            "imf": imf_h,
            "maskcs": maskcs,
            "clsb": clsb,
        })
    return MT, in_maps


def get_program(MT: int):
    if MT not in _kernel_cache:
        _kernel_cache[MT] = _build(MT)
    return _kernel_cache[MT]


def kernel(audio_feats, image_feats, audio_cls, image_cls, audio_mask, agg_heads):
    global LAST_RESULTS
    MT, in_maps = prepare_inputs(
        audio_feats, image_feats, audio_cls, image_cls, audio_mask
    )
    nc = get_program(MT)
    res = run_bass_kernel_spmd(nc, in_maps, list(range(NCORES)), trace=TRACE)
    LAST_RESULTS = res
    agg = bool(np.asarray(agg_heads))
    outs = []
    for ci in range(NCORES):
        if agg:
            outs.append(res.results[ci]["outsum"])  # (A, VL)
        else:
            outk = res.results[ci]["outk"].reshape(A, K, VL)
            outs.append(outk.transpose(0, 2, 1))    # (A, VL, K)
    return np.concatenate(outs, axis=1).astype(np.float32)
